# revision 1
# baseline (speedup 1.0000x reference)
"""Trainium2 Bass kernel for nn_CBAMSpaceMask (CBAM spatial mask over T timestep blocks).

Math per timestep block t (3 channels):
  mx_c = maxpool3x3(x_c)          (stride 1, -inf pad == replicate pad)
  av_c = avgpool3x3(x_c)/9        (zero pad, count_include_pad)
  y_t  = sum_c wM_c * mx_c + wA_c * av_c + b   (3x3 conv, zero pad)
  out[3t+c] = sigmoid(leakyrelu(y_t))          (broadcast over c)

Device decomposition (per core = 1 batch):
  - pools on DVE in bf16; vertical max via 3 row-shifted HBM loads (partition-aligned)
  - avg path: bh = horizontal box sum (DVE), vertical part folded into the conv operator
  - conv: banded-Toeplitz matmuls on PE (bf16): y[:, w'] += OP[path,c,kw] @ P[:, w'+kw-1]
    with exact boundary handling baked into host-precomputed operator matrices
  - epilogue: ACT Identity(+bias) -> DVE leaky(max(v, .01v)) -> ACT Sigmoid -> 3x DMA out
Sharding: pure data parallel, batch dim across 8 cores.
"""
import sys

sys.path.insert(0, "/opt/trn_rl_repo")

import numpy as np
import ml_dtypes
from contextlib import ExitStack

import concourse.bass as bass
import concourse.tile as tile
from concourse import bacc, mybir
from concourse.bass_utils import run_bass_kernel_spmd

F32 = mybir.dt.float32
BF16 = mybir.dt.bfloat16

B, CTOT, H, W = 8, 48, 256, 256
T = 16
N_CORES = 8
PLANES = CTOT  # 48 planes per core
NGRP = 6       # 8 planes per group
# conv chunk geometry: (m0, m1, r0, r1): y rows [m0,m1) from input rows [r0,r1)
CHUNKS = [(0, 124, 0, 128), (124, 248, 122, 250), (248, 256, 246, 256)]
NMAT = 2 * 3 * 3 * 3  # path, c, kw, chunk

_cache = {}

import os
DBG_SKIP_T2 = os.environ.get("DBG_SKIP_T2", "0") == "1"     # drop chunk-2 + t2 tiles
DBG_MAX_TS = int(os.environ.get("DBG_MAX_TS", "16"))        # limit timesteps
DBG_MAX_GRP = int(os.environ.get("DBG_MAX_GRP", "6"))       # limit groups


def _build_stack(conv_w):
    """lhsT stack [128, NMAT, 128] f32: mats[path][c][kw][chunk] = OP[m0:m1, r0:r1].T.

    chunk-2 (K=10, M=8) matrices are replicated at partition bases 0/32/64 so the
    matmul base-partition pairing (lhsT base == rhs base) works for the packed
    last-rows tile.
    """
    w = conv_w[0].astype(np.float64)  # [6, 3, 3]
    eye = np.eye(H)
    Bv = np.zeros((H, H))
    for i in (-1, 0, 1):
        Bv += np.eye(H, k=i)
    stack = np.zeros((128, NMAT, 128), dtype=np.float64)
    idx = 0
    for path in range(2):  # 0 = max (mx planes), 1 = avg (bh planes)
        for c in range(3):
            for kw in range(3):
                op = np.zeros((H, H))
                k2d = w[2 * c] if path == 0 else w[2 * c + 1]
                for kh in range(3):
                    op += k2d[kh, kw] * np.eye(H, k=kh - 1)
                if path == 1:
                    op = (op @ Bv) / 9.0
                for ch, (m0, m1, r0, r1) in enumerate(CHUNKS):
                    lhsT = op[m0:m1, r0:r1].T  # [K, M]
                    K, M = lhsT.shape
                    mat = idx * 3 + ch
                    if ch < 2:
                        stack[:K, mat, :M] = lhsT
                    else:
                        for base in (0, 32, 64):
                            stack[base:base + K, mat, :M] = lhsT
                idx += 1
    return stack.astype(ml_dtypes.bfloat16)


def _mat_index(path, c, kw, ch):
    return ((path * 3 + c) * 3 + kw) * 3 + ch


def _build_program():
    nc = bacc.Bacc("TRN2", target_bir_lowering=False, debug=False, enable_asserts=False)
    x_ap = nc.dram_tensor("x", [PLANES, H, W], F32, kind="ExternalInput").ap()
    cst_ap = nc.dram_tensor("cst", [128, NMAT, 128], BF16, kind="ExternalInput").ap()
    bias_ap = nc.dram_tensor("bias", [128, 1], F32, kind="ExternalInput").ap()
    out_ap = nc.dram_tensor("out", [PLANES, H, W], F32, kind="ExternalOutput").ap()

    with tile.TileContext(nc) as tc, ExitStack() as ctx:
        const_pool = ctx.enter_context(tc.tile_pool(name="const", bufs=1))
        psum_pool = ctx.enter_context(tc.tile_pool(name="psum", bufs=4, space="PSUM"))
        epi_pool = ctx.enter_context(tc.tile_pool(name="epi", bufs=3))

        cst = const_pool.tile([128, NMAT, 128], BF16, tag="cst")
        nc.sync.dma_start(out=cst[:], in_=cst_ap)
        bias = const_pool.tile([128, 1], F32, tag="bias")
        nc.sync.dma_start(out=bias[:], in_=bias_ap)

        MAXOP = mybir.AluOpType.max
        ADDOP = mybir.AluOpType.add

        def pools(tmp_pool, X, U, D, mx, bh, p):
            """3x3 maxpool (X/U/D row-shifted) + horizontal box sum, partitions [0:p)."""
            b = tmp_pool.tile(list(X.shape), BF16, tag="poolb")
            vx = tmp_pool.tile(list(X.shape), BF16, tag="poolvx")
            nc.vector.tensor_tensor(out=b[:p], in0=U[:p], in1=D[:p], op=MAXOP)
            nc.vector.tensor_tensor(out=vx[:p], in0=b[:p], in1=X[:p], op=MAXOP)
            # horizontal max into mx
            nc.vector.tensor_tensor(out=mx[:p, :, 0:255], in0=vx[:p, :, 0:255],
                                    in1=vx[:p, :, 1:256], op=MAXOP)
            nc.vector.tensor_copy(mx[:p, :, 255:256], vx[:p, :, 255:256])
            nc.vector.tensor_tensor(out=mx[:p, :, 1:256], in0=mx[:p, :, 1:256],
                                    in1=vx[:p, :, 0:255], op=MAXOP)
            # horizontal box sum of X into bh (zero pad)
            nc.vector.tensor_tensor(out=bh[:p, :, 0:255], in0=X[:p, :, 0:255],
                                    in1=X[:p, :, 1:256], op=ADDOP)
            nc.vector.tensor_copy(bh[:p, :, 255:256], X[:p, :, 255:256])
            nc.vector.tensor_tensor(out=bh[:p, :, 1:256], in0=bh[:p, :, 1:256],
                                    in1=X[:p, :, 0:255], op=ADDOP)

        KW_ORDER = [(path, c, kw)
                    for c in range(3) for path in range(2) for kw in (1, 0, 2)]
        KW_ORDER.sort(key=lambda pck: 0 if pck[2] == 1 else 1)

        def conv_chunk(t, ch, rhs_fn):
            """One psum accumulation (18 matmuls) + epilogue + 3 output DMAs."""
            m0, m1, r0, r1 = CHUNKS[ch]
            M = m1 - m0
            K = r1 - r0
            ps = psum_pool.tile([128, W], F32, tag="ps")
            n = len(KW_ORDER)
            for i, (path, c, kw) in enumerate(KW_ORDER):
                q = 3 * t + c
                s = kw - 1
                lo, hi = max(0, -s), W - max(0, s)
                mat = _mat_index(path, c, kw, ch)
                rhs = rhs_fn(path, q, K, lo + s, hi + s)
                lhsT = cst[0:K, mat, 0:M]
                nc.tensor.matmul(ps[0:M, lo:hi], lhsT, rhs,
                                 start=(i == 0), stop=(i == n - 1))
            # epilogue: v = psum + bias ; leaky = max(v, .01v) ; sigmoid
            v = epi_pool.tile([128, W], F32, tag="epiv")
            nc.scalar.activation(v[0:M], ps[0:M],
                                 mybir.ActivationFunctionType.Identity,
                                 bias=bias[0:M], scale=1.0)
            lk = epi_pool.tile([128, W], F32, tag="epil")
            nc.vector.scalar_tensor_tensor(out=lk[0:M], in0=v[0:M], scalar=0.01,
                                           in1=v[0:M], op0=mybir.AluOpType.mult,
                                           op1=MAXOP)
            sg = epi_pool.tile([128, W], F32, tag="epis")
            nc.scalar.activation(sg[0:M], lk[0:M],
                                 mybir.ActivationFunctionType.Sigmoid)
            for c in range(3):
                nc.sync.dma_start(out=out_ap[3 * t + c, m0:m1, :], in_=sg[0:M])

        # ---- phase A: chunk 2 (last 8 y-rows) on [10, 48, W] base-0 tiles.
        # All 48 planes packed in the free dim; partition = x row 246+r.
        if not DBG_SKIP_T2:
            with tc.tile_pool(name="t2", bufs=1) as t2_pool:
                X = t2_pool.tile([10, PLANES, W], BF16, tag="t2x")
                U = t2_pool.tile([10, PLANES, W], BF16, tag="t2u")
                D = t2_pool.tile([10, PLANES, W], BF16, tag="t2d")
                nc.gpsimd.dma_start(out=X[:], in_=x_ap[:, 246:256, :].transpose([1, 0, 2]))
                nc.gpsimd.dma_start(out=D[:], in_=x_ap[:, 245:255, :].transpose([1, 0, 2]))
                nc.gpsimd.dma_start(out=U[0:9], in_=x_ap[:, 247:256, :].transpose([1, 0, 2]))
                nc.gpsimd.dma_start(out=U[9:10], in_=x_ap[:, 255:256, :].transpose([1, 0, 2]))
                mx2 = t2_pool.tile([10, PLANES, W], BF16, tag="t2mx")
                bh2 = t2_pool.tile([10, PLANES, W], BF16, tag="t2bh")
                pools(t2_pool, X, U, D, mx2, bh2, 10)

                def rhs_t2(path, q, K, wlo, whi):
                    src = mx2 if path == 0 else bh2
                    return src[0:K, q, wlo:whi]

                for t in range(min(T, DBG_MAX_TS)):
                    conv_chunk(t, 2, rhs_t2)

        # ---- phase B: main groups, subtiles t0 rows [0:128), t1 rows [122:250)
        with tc.tile_pool(name="loads", bufs=2) as ld_pool, \
             tc.tile_pool(name="ptmp", bufs=2) as pool_tmp, \
             tc.tile_pool(name="mxbh", bufs=2) as mxbh_pool:
            mx_tiles = {}  # (g, sub) -> tile
            bh_tiles = {}

            def load_group(g):
                src = x_ap[8 * g:8 * g + 8]
                for sub, (r0, r1) in enumerate([(0, 128), (122, 250)]):
                    X = ld_pool.tile([128, 8, W], BF16, tag=f"ldx{sub}")
                    U = ld_pool.tile([128, 8, W], BF16, tag=f"ldu{sub}")
                    D = ld_pool.tile([128, 8, W], BF16, tag=f"ldd{sub}")
                    nc.gpsimd.dma_start(out=X[:], in_=src[:, r0:r1, :].transpose([1, 0, 2]))
                    nc.gpsimd.dma_start(out=U[:], in_=src[:, r0 + 1:r1 + 1, :].transpose([1, 0, 2]))
                    if sub == 0:
                        nc.gpsimd.dma_start(out=D[1:128],
                                            in_=src[:, 0:127, :].transpose([1, 0, 2]))
                        nc.gpsimd.dma_start(out=D[0:1],
                                            in_=src[:, 0:1, :].transpose([1, 0, 2]))
                    else:
                        nc.gpsimd.dma_start(out=D[:], in_=src[:, r0 - 1:r1 - 1, :].transpose([1, 0, 2]))
                    mx = mxbh_pool.tile([128, 8, W], BF16, tag=f"mx{sub}")
                    bh = mxbh_pool.tile([128, 8, W], BF16, tag=f"bh{sub}")
                    pools(pool_tmp, X, U, D, mx, bh, 128)
                    mx_tiles[(g, sub)] = mx
                    bh_tiles[(g, sub)] = bh

            def rhs_main(ch):
                def f(path, q, K, wlo, whi):
                    g, pl = q // 8, q % 8
                    srcs = mx_tiles if path == 0 else bh_tiles
                    return srcs[(g, ch)][0:K, pl, wlo:whi]
                return f

            done_ts = 0
            for g in range(min(NGRP, DBG_MAX_GRP)):
                load_group(g)
                ready_ts = min(T, (8 * g + 8) // 3, DBG_MAX_TS)
                for t in range(done_ts, ready_ts):
                    conv_chunk(t, 0, rhs_main(0))
                    conv_chunk(t, 1, rhs_main(1))
                done_ts = ready_ts

    nc.compile()
    return nc


def kernel(input_tensor, conv_w, conv_b):
    input_tensor = np.ascontiguousarray(np.asarray(input_tensor, dtype=np.float32))
    conv_w = np.asarray(conv_w, dtype=np.float32)
    conv_b = np.asarray(conv_b, dtype=np.float32)

    if "nc" not in _cache:
        _cache["nc"] = _build_program()
    nc = _cache["nc"]

    stack = _build_stack(conv_w)
    bias_vec = np.full((128, 1), conv_b[0], dtype=np.float32)
    in_maps = [
        {"x": input_tensor[i], "cst": stack, "bias": bias_vec}
        for i in range(N_CORES)
    ]
    res = run_bass_kernel_spmd(nc, in_maps, list(range(N_CORES)))
    out = np.stack([res.results[i]["out"] for i in range(N_CORES)], axis=0)
    return out.astype(np.float32)


if __name__ == "__main__":
    rng = np.random.default_rng(0)
    x = rng.standard_normal((B, CTOT, H, W), dtype=np.float32)
    cw = rng.uniform(-0.1, 0.1, (1, 6, 3, 3)).astype(np.float32)
    cb = np.array([0.01], dtype=np.float32)
    o = kernel(x, cw, cb)
    print(o.shape, o.dtype)

